# revision 6
# baseline (speedup 1.0000x reference)
"""Cross-attention kernel for Trainium2, data-parallel over batch across 8 NeuronCores.

Reference computation (per batch element b):
    q = Wq @ sem_b   [64, 4096]   (1x1 conv == per-pixel linear)
    k = Wk @ foren_b [64, 4096]
    v = Wv @ foren_b [256, 4096]
    S = (q^T k) / 8                [4096 (n), 4096 (m)]
    P = softmax_m(S)
    out = v @ P^T                  [256, 4096]
    y = sem_b + gamma * out

Kernel layout strategy (keys m live on partitions, so softmax needs no on-chip
transposes and the denominator rides the value matmul):
    - S^T[m, n] = sum_d k[d, m] q[d, n]: lhsT = k tile (K=64), rhs = q chunk.
      q/k are duplicated into partitions 64..127 so two K=64 matmuls run
      concurrently in the two halves of the PE array via tile_position.
    - P~ = exp(S^T/8) (no max subtraction: logits ~ N(0,1), exp is safe in fp32),
      ScalarE reads PSUM, writes bf16.
    - v^T[m, c] = sum_ch foren[ch, m] WvT[ch, c]: natural from PE; gamma folded
      into the eviction; a ones column is appended -> vt_ext [m, 257].
    - out^T[n, c'] = sum_m P~[m, n-slice] vt_ext[m, c']: lhsT = P~ slice
      (stationary), rhs = vt_ext. Column 256 of the accumulator is then
      sum_m P~[m, n] — the softmax denominator, for free.
    - Eviction: rinv = 1/acc[:, 256] (per-partition!), ScalarE Copy with
      scale=rinv normalizes while casting to bf16.
    - PE transpose (128x128, via identity) flips out^T back to [c, n];
      VectorE adds the residual sem + gamma*bv (bv folded there since
      out = v@P^T + bv when v carries bias).
"""

import os
import sys

for _p in ("/opt/trn_rl_repo",):
    if _p not in sys.path and os.path.isdir(_p):
        sys.path.append(_p)

import numpy as np

import concourse.bass as bass
import concourse.tile as tile
from concourse import bacc, mybir
from concourse.bass_utils import run_bass_kernel_spmd
from concourse.masks import make_identity

N_CORES = 8
DIM = 256
D4 = 64
HW = 4096
P = 128
NCHUNK = 512          # n per chunk
NCHUNKS = HW // NCHUNK
MTILES = HW // P      # 32 key tiles of 128
VTW = 264             # vt_ext row stride (257 used, padded for alignment)

TRACE = False         # test.py sets this to capture HW exec time
_CACHE = {}


def _build_program():
    f32 = mybir.dt.float32
    bf16 = mybir.dt.bfloat16
    AF = mybir.ActivationFunctionType
    ALU = mybir.AluOpType

    nc = bacc.Bacc("TRN2", target_bir_lowering=False, debug=False,
                   num_devices=N_CORES)

    sem_d = nc.dram_tensor("sem_b", [DIM, HW], f32, kind="ExternalInput")
    foren_d = nc.dram_tensor("foren_b", [DIM, HW], f32, kind="ExternalInput")
    wqt_d = nc.dram_tensor("WqT", [DIM, D4], f32, kind="ExternalInput")
    wkt_d = nc.dram_tensor("WkT", [DIM, D4], f32, kind="ExternalInput")
    wvt_d = nc.dram_tensor("WvT", [DIM, DIM], f32, kind="ExternalInput")
    bq_d = nc.dram_tensor("bq2", [P, 1], f32, kind="ExternalInput")
    bk_d = nc.dram_tensor("bk2", [P, 1], f32, kind="ExternalInput")
    bv_d = nc.dram_tensor("bv2", [DIM, 1], f32, kind="ExternalInput")
    gamma_d = nc.dram_tensor("gamma2", [1, 1], f32, kind="ExternalInput")
    y_d = nc.dram_tensor("y", [DIM, HW], f32, kind="ExternalOutput")

    with tile.TileContext(nc) as tc:
        with (
            tc.tile_pool(name="persist", bufs=1) as persist,
            tc.tile_pool(name="big", bufs=2) as big,        # foren_bf then P~ tiles
            tc.tile_pool(name="stage", bufs=3) as stage,    # fp32 DMA staging
            tc.tile_pool(name="evict", bufs=4) as evict,
            tc.tile_pool(name="st_ps", bufs=2, space="PSUM") as st_ps,
            tc.tile_pool(name="oacc_ps", bufs=4, space="PSUM") as oacc_ps,
        ):
            # ---- load inputs (sem on sync queue, foren on scalar queue) ----
            sem = [persist.tile([P, HW], f32, tag=f"sem{t}", name=f"sem{t}")
                   for t in range(2)]
            for t in range(2):
                nc.sync.dma_start(sem[t][:], sem_d[t * P:(t + 1) * P, :])

            foren_bf = [big.tile([P, HW], bf16, tag="big", name=f"forenb{t}")
                        for t in range(2)]
            for t in range(2):
                for h in range(2):
                    hs = slice(h * (HW // 2), (h + 1) * (HW // 2))
                    fst = stage.tile([P, HW // 2], f32, tag="fst", name="fst")
                    nc.scalar.dma_start(fst[:], foren_d[t * P:(t + 1) * P, hs])
                    nc.vector.tensor_copy(foren_bf[t][:, hs], fst[:])

            wqt = persist.tile([P, 2, D4], f32, tag="wqt")
            wkt = persist.tile([P, 2, D4], f32, tag="wkt")
            wvt = persist.tile([P, 2, DIM], f32, tag="wvt")
            nc.gpsimd.dma_start(wqt[:], wqt_d.ap().rearrange("(t p) o -> p t o", p=P))
            nc.gpsimd.dma_start(wkt[:], wkt_d.ap().rearrange("(t p) o -> p t o", p=P))
            nc.gpsimd.dma_start(wvt[:], wvt_d.ap().rearrange("(t p) o -> p t o", p=P))

            bq = persist.tile([P, 1], f32, tag="bq")
            bk = persist.tile([P, 1], f32, tag="bk")
            bv = persist.tile([P, 2, 1], f32, tag="bv")
            gamma = persist.tile([P, 1], f32, tag="gamma")
            nc.gpsimd.dma_start(bq[:], bq_d[:])
            nc.gpsimd.dma_start(bk[:], bk_d[:])
            nc.gpsimd.dma_start(bv[:], bv_d.ap().rearrange("(t p) o -> p t o", p=P))
            nc.gpsimd.dma_start(gamma[:], gamma_d.ap().to_broadcast([P, 1]))

            # ---- bf16 casts ----
            sem_bf = [persist.tile([P, HW], bf16, tag=f"semb{t}", name=f"semb{t}")
                      for t in range(2)]
            for t in range(2):
                nc.vector.tensor_copy(sem_bf[t][:], sem[t][:])
            wqt_bf = persist.tile([P, 2, D4], bf16, tag="wqtb")
            wkt_bf = persist.tile([P, 2, D4], bf16, tag="wktb")
            wvt_bf = persist.tile([P, 2, DIM], bf16, tag="wvtb")
            nc.vector.tensor_copy(wqt_bf[:], wqt[:])
            nc.vector.tensor_copy(wkt_bf[:], wkt[:])
            nc.vector.tensor_copy(wvt_bf[:], wvt[:])

            identity_bf = persist.tile([P, P], bf16, tag="idn")
            make_identity(nc, identity_bf[:])

            # ---- q/k projections into lower partitions of q2/k2 ----
            q2 = persist.tile([P, HW], bf16, tag="q2")
            k2 = persist.tile([P, HW], bf16, tag="k2")
            # each projection matmul is issued twice with col-tiling so the
            # result lands in both partition halves of PSUM concurrently --
            # q2/k2 come out pre-duplicated for the row-packed QK matmuls
            for j in range(NCHUNKS):
                ns = slice(j * NCHUNK, (j + 1) * NCHUNK)
                pq = st_ps.tile([P, NCHUNK], f32, tag="st", name="pq")
                pk = st_ps.tile([P, NCHUNK], f32, tag="st", name="pk")
                for t in range(2):
                    for cg in range(2):
                        ps_ = slice(cg * D4, (cg + 1) * D4)
                        nc.tensor.matmul(pq[ps_, :], wqt_bf[:, t, :],
                                         sem_bf[t][:, ns],
                                         start=(t == 0), stop=(t == 1),
                                         tile_position=(0, cg * D4))
                        nc.tensor.matmul(pk[ps_, :], wkt_bf[:, t, :],
                                         foren_bf[t][:, ns],
                                         start=(t == 0), stop=(t == 1),
                                         tile_position=(0, cg * D4))
                nc.vector.tensor_scalar_add(q2[:, ns], pq[:], bq[:])
                nc.vector.tensor_scalar_add(k2[:, ns], pk[:], bk[:])

            # ---- vT projection: vt_ext[m, c] = gamma * (Wv@foren)^T; col 256 = 1 ----
            vt_ext = persist.tile([P, MTILES, VTW], bf16, tag="vte")
            nc.vector.memset(vt_ext[:, :, DIM:DIM + 1], 1.0)
            for mi in range(MTILES):
                ms = slice(mi * P, (mi + 1) * P)
                pv = st_ps.tile([P, DIM], f32, tag="st", name="pv")
                for t in range(2):
                    nc.tensor.matmul(pv[:], foren_bf[t][:, ms], wvt_bf[:, t, :],
                                     start=(t == 0), stop=(t == 1))
                nc.vector.tensor_scalar_mul(vt_ext[:, mi, 0:DIM], pv[:], gamma[:])

            # ---- residual base: sem_res = sem + gamma * bv (in place) ----
            bvg = persist.tile([P, 2, 1], f32, tag="bvg")
            nc.vector.tensor_tensor(bvg[:, 0, :], bv[:, 0, :], gamma[:], op=ALU.mult)
            nc.vector.tensor_tensor(bvg[:, 1, :], bv[:, 1, :], gamma[:], op=ALU.mult)
            for t in range(2):
                nc.vector.tensor_scalar_add(sem[t][:], sem[t][:], bvg[:, t, :])

            # ---- main attention loop over n-chunks ----
            for j in range(NCHUNKS):
                ns = slice(j * NCHUNK, (j + 1) * NCHUNK)
                pt = big.tile([P, MTILES // 2, 2 * NCHUNK], bf16, tag="big",
                              name="pt")
                y_stage = evict.tile([P, 2, NCHUNK], f32, tag="ystage",
                                     name="ystage", bufs=2)
                for mp in range(MTILES // 2):
                    st = st_ps.tile([P, 2 * NCHUNK], f32, tag="st", name="st")
                    m0, m1 = 2 * mp, 2 * mp + 1
                    nc.tensor.matmul(
                        st[:, 0:NCHUNK],
                        k2[0:D4, m0 * P:(m0 + 1) * P], q2[0:D4, ns],
                        start=True, stop=True, tile_position=(0, 0))
                    nc.tensor.matmul(
                        st[:, NCHUNK:2 * NCHUNK],
                        k2[D4:P, m1 * P:(m1 + 1) * P], q2[D4:P, ns],
                        start=True, stop=True, tile_position=(64, 0))
                    nc.scalar.activation(pt[:, mp, :], st[:], AF.Exp, scale=0.125)

                for t in range(NCHUNK // P):
                    oacc = oacc_ps.tile([P, DIM + 1], f32, tag="oacc", name="oacc")
                    for m in range(MTILES):
                        off = (m % 2) * NCHUNK + t * P
                        nc.tensor.matmul(
                            oacc[:],
                            pt[:, m // 2, off:off + P],
                            vt_ext[:, m, 0:DIM + 1],
                            start=(m == 0), stop=(m == MTILES - 1))
                    rinv = evict.tile([P, 1], f32, tag="rinv", name="rinv")
                    nc.vector.reciprocal(rinv[:], oacc[:, DIM:DIM + 1])
                    onrm = evict.tile([P, DIM], bf16, tag="onrm", name="onrm")
                    nc.vector.tensor_scalar_mul(onrm[:], oacc[:, 0:DIM], rinv[:])
                    gs = slice(j * NCHUNK + t * P, j * NCHUNK + (t + 1) * P)
                    for ct in range(2):
                        tp = evict.tile([P, P], bf16, tag="tp", name="tp")
                        nc.sync.dma_start_transpose(tp[:],
                                                    onrm[:, ct * P:(ct + 1) * P])
                        nc.vector.tensor_tensor(
                            y_stage[:, ct, t * P:(t + 1) * P], tp[:],
                            sem[ct][:, gs], op=ALU.add)
                for ct in range(2):
                    nc.scalar.dma_start(y_d[ct * P:(ct + 1) * P, ns],
                                        y_stage[:, ct, :])

    nc.compile()
    return nc


def _get_program():
    if "nc" not in _CACHE:
        _CACHE["nc"] = _build_program()
    return _CACHE["nc"]


def kernel(sem, foren, Wq, bq, Wk, bk, Wv, bv, gamma):
    sem = np.asarray(sem, dtype=np.float32)
    foren = np.asarray(foren, dtype=np.float32)
    wqt = np.ascontiguousarray(np.asarray(Wq, np.float32).T)
    wkt = np.ascontiguousarray(np.asarray(Wk, np.float32).T)
    wvt = np.ascontiguousarray(np.asarray(Wv, np.float32).T)
    bq2 = np.ascontiguousarray(np.tile(np.asarray(bq, np.float32).reshape(D4, 1), (2, 1)))
    bk2 = np.ascontiguousarray(np.tile(np.asarray(bk, np.float32).reshape(D4, 1), (2, 1)))
    bv2 = np.ascontiguousarray(np.asarray(bv, np.float32).reshape(DIM, 1))
    g2 = np.ascontiguousarray(np.asarray(gamma, np.float32).reshape(1, 1))

    B = sem.shape[0]
    assert B == N_CORES, f"expected batch {N_CORES}, got {B}"

    in_maps = []
    for i in range(N_CORES):
        in_maps.append({
            "sem_b": np.ascontiguousarray(sem[i].reshape(DIM, HW)),
            "foren_b": np.ascontiguousarray(foren[i].reshape(DIM, HW)),
            "WqT": wqt, "WkT": wkt, "WvT": wvt,
            "bq2": bq2, "bk2": bk2, "bv2": bv2, "gamma2": g2,
        })

    nc = _get_program()
    res = run_bass_kernel_spmd(nc, in_maps, list(range(N_CORES)), trace=TRACE)
    if TRACE:
        _CACHE["last_exec_time_ns"] = res.exec_time_ns
        _CACHE["last_results"] = res

    H = int(np.sqrt(HW))
    out = np.stack([res.results[i]["y"].reshape(DIM, H, H)
                    for i in range(N_CORES)])
    return out.astype(np.float32)


# revision 11
# speedup vs baseline: 1.1925x; 1.1925x over previous
"""Cross-attention kernel for Trainium2, data-parallel over batch across 8 NeuronCores.

Reference computation (per batch element b):
    q = Wq @ sem_b   [64, 4096]   (1x1 conv == per-pixel linear)
    k = Wk @ foren_b [64, 4096]
    v = Wv @ foren_b [256, 4096]
    S = (q^T k) / 8                [4096 (n), 4096 (m)]
    P = softmax_m(S)
    out = v @ P^T                  [256, 4096]
    y = sem_b + gamma * out

Kernel layout strategy (keys m live on partitions, so softmax needs no on-chip
transposes and the denominator rides the value matmul):
    - S^T[m, n] = sum_d k[d, m] q[d, n]: lhsT = k tile (K=64), rhs = q chunk.
      q/k are duplicated into partitions 64..127 so two K=64 matmuls run
      concurrently in the two halves of the PE array via tile_position.
    - P~ = exp(S^T/8) (no max subtraction: logits ~ N(0,1), exp is safe in fp32),
      ScalarE reads PSUM, writes bf16.
    - v^T[m, c] = sum_ch foren[ch, m] WvT[ch, c]: natural from PE; gamma folded
      into the eviction; a ones column is appended -> vt_ext [m, 257].
    - out^T[n, c'] = sum_m P~[m, n-slice] vt_ext[m, c']: lhsT = P~ slice
      (stationary), rhs = vt_ext. Column 256 of the accumulator is then
      sum_m P~[m, n] — the softmax denominator, for free.
    - Eviction: rinv = 1/acc[:, 256] (per-partition!), ScalarE Copy with
      scale=rinv normalizes while casting to bf16.
    - PE transpose (128x128, via identity) flips out^T back to [c, n];
      VectorE adds the residual sem + gamma*bv (bv folded there since
      out = v@P^T + bv when v carries bias).
"""

import os
import sys

for _p in ("/opt/trn_rl_repo",):
    if _p not in sys.path and os.path.isdir(_p):
        sys.path.append(_p)

import numpy as np

import concourse.bass as bass
import concourse.tile as tile
from concourse import bacc, mybir
from concourse.bass_utils import run_bass_kernel_spmd
from concourse.masks import make_identity

N_CORES = 8
DIM = 256
D4 = 64
HW = 4096
P = 128
NCHUNK = 512          # n per chunk
NCHUNKS = HW // NCHUNK
MTILES = HW // P      # 32 key tiles of 128
VTW = 264             # vt_ext row stride (257 used, padded for alignment)

TRACE = False         # test.py sets this to capture HW exec time
_CACHE = {}


def _build_program():
    f32 = mybir.dt.float32
    bf16 = mybir.dt.bfloat16
    AF = mybir.ActivationFunctionType
    ALU = mybir.AluOpType

    nc = bacc.Bacc("TRN2", target_bir_lowering=False, debug=False,
                   num_devices=N_CORES)

    sem_d = nc.dram_tensor("sem_b", [DIM, HW], f32, kind="ExternalInput")
    foren_d = nc.dram_tensor("foren_b", [DIM, HW], f32, kind="ExternalInput")
    wqt_d = nc.dram_tensor("WqT", [DIM, D4], f32, kind="ExternalInput")
    wkt_d = nc.dram_tensor("WkT", [DIM, D4], f32, kind="ExternalInput")
    wvt_d = nc.dram_tensor("WvT", [DIM, DIM], f32, kind="ExternalInput")
    bq_d = nc.dram_tensor("bq2", [P, 1], f32, kind="ExternalInput")
    bk_d = nc.dram_tensor("bk2", [P, 1], f32, kind="ExternalInput")
    bv_d = nc.dram_tensor("bv2", [DIM, 1], f32, kind="ExternalInput")
    gamma_d = nc.dram_tensor("gamma2", [1, 1], f32, kind="ExternalInput")
    y_d = nc.dram_tensor("y", [DIM, HW], f32, kind="ExternalOutput")

    with tile.TileContext(nc) as tc:
        with (
            tc.tile_pool(name="persist", bufs=1) as persist,
            tc.tile_pool(name="big", bufs=2) as big,        # foren_bf then P~ tiles
            tc.tile_pool(name="stage", bufs=3) as stage,    # fp32 DMA staging
            tc.tile_pool(name="evict", bufs=4) as evict,
            tc.tile_pool(name="st_ps", bufs=2, space="PSUM") as st_ps,
            tc.tile_pool(name="oacc_ps", bufs=2, space="PSUM") as oacc_ps,
            tc.tile_pool(name="tp_ps", bufs=2, space="PSUM") as tp_ps,
        ):
            # ---- load inputs (sem on sync queue, foren on scalar queue) ----
            sem = [persist.tile([P, HW], f32, tag=f"sem{t}", name=f"sem{t}")
                   for t in range(2)]
            for t in range(2):
                nc.sync.dma_start(sem[t][:], sem_d[t * P:(t + 1) * P, :])

            foren_bf = [big.tile([P, HW], bf16, tag="big", name=f"forenb{t}")
                        for t in range(2)]
            for t in range(2):
                for h in range(2):
                    hs = slice(h * (HW // 2), (h + 1) * (HW // 2))
                    fst = stage.tile([P, HW // 2], f32, tag="fst", name="fst")
                    nc.scalar.dma_start(fst[:], foren_d[t * P:(t + 1) * P, hs])
                    nc.vector.tensor_copy(foren_bf[t][:, hs], fst[:])

            wqt = persist.tile([P, 2, D4], f32, tag="wqt")
            wkt = persist.tile([P, 2, D4], f32, tag="wkt")
            wvt = persist.tile([P, 2, DIM], f32, tag="wvt")
            nc.gpsimd.dma_start(wqt[:], wqt_d.ap().rearrange("(t p) o -> p t o", p=P))
            nc.gpsimd.dma_start(wkt[:], wkt_d.ap().rearrange("(t p) o -> p t o", p=P))
            nc.gpsimd.dma_start(wvt[:], wvt_d.ap().rearrange("(t p) o -> p t o", p=P))

            bq = persist.tile([P, 1], f32, tag="bq")
            bk = persist.tile([P, 1], f32, tag="bk")
            bv = persist.tile([P, 2, 1], f32, tag="bv")
            gamma = persist.tile([P, 1], f32, tag="gamma")
            nc.gpsimd.dma_start(bq[:], bq_d[:])
            nc.gpsimd.dma_start(bk[:], bk_d[:])
            nc.gpsimd.dma_start(bv[:], bv_d.ap().rearrange("(t p) o -> p t o", p=P))
            nc.gpsimd.dma_start(gamma[:], gamma_d.ap().to_broadcast([P, 1]))

            # ---- bf16 casts ----
            sem_bf = [persist.tile([P, HW], bf16, tag=f"semb{t}", name=f"semb{t}")
                      for t in range(2)]
            for t in range(2):
                nc.scalar.copy(sem_bf[t][:], sem[t][:])
            wqt_bf = persist.tile([P, 2, D4], bf16, tag="wqtb")
            wkt_bf = persist.tile([P, 2, D4], bf16, tag="wktb")
            wvt_bf = persist.tile([P, 2, DIM], bf16, tag="wvtb")
            nc.vector.tensor_copy(wqt_bf[:], wqt[:])
            nc.vector.tensor_copy(wkt_bf[:], wkt[:])
            nc.vector.tensor_copy(wvt_bf[:], wvt[:])

            identity_bf = persist.tile([P, P], bf16, tag="idn")
            make_identity(nc, identity_bf[:])

            # ---- q/k projections into lower partitions of q2/k2 ----
            q2 = persist.tile([P, HW], bf16, tag="q2")
            k2 = persist.tile([P, HW], bf16, tag="k2")
            vt_ext = persist.tile([P, MTILES, VTW], bf16, tag="vte")
            nc.vector.memset(vt_ext[:, :, DIM:DIM + 1], 1.0)
            # each projection matmul is issued twice with col-tiling so the
            # result lands in both partition halves of PSUM concurrently --
            # q2/k2 come out pre-duplicated for the row-packed QK matmuls
            for j in range(NCHUNKS):
                ns = slice(j * NCHUNK, (j + 1) * NCHUNK)
                pq = st_ps.tile([P, NCHUNK], f32, tag="st", name="pq")
                pk = st_ps.tile([P, NCHUNK], f32, tag="st", name="pk")
                for t in range(2):
                    for cg in range(2):
                        ps_ = slice(cg * D4, (cg + 1) * D4)
                        nc.tensor.matmul(pq[ps_, :], wqt_bf[:, t, :],
                                         sem_bf[t][:, ns],
                                         start=(t == 0), stop=(t == 1),
                                         tile_position=(0, cg * D4))
                        nc.tensor.matmul(pk[ps_, :], wkt_bf[:, t, :],
                                         foren_bf[t][:, ns],
                                         start=(t == 0), stop=(t == 1),
                                         tile_position=(0, cg * D4))
                nc.scalar.activation(q2[:, ns], pq[:], AF.Identity, bias=bq[:])
                nc.scalar.activation(k2[:, ns], pk[:], AF.Identity, bias=bk[:])
                # interleave vT projection (evicted on VectorE, own PSUM
                # rotation) so setup PSUM slots recycle through two engines
                for mi in range(4 * j, 4 * j + 4):
                    ms = slice(mi * P, (mi + 1) * P)
                    pv = oacc_ps.tile([P, DIM], f32, tag="oacc", name="pv")
                    for t in range(2):
                        nc.tensor.matmul(pv[:], foren_bf[t][:, ms],
                                         wvt_bf[:, t, :],
                                         start=(t == 0), stop=(t == 1))
                    nc.vector.tensor_scalar_mul(vt_ext[:, mi, 0:DIM], pv[:],
                                                gamma[:])

            # ---- residual base: sem_res = sem + gamma * bv (in place) ----
            bvg = persist.tile([P, 2, 1], f32, tag="bvg")
            nc.vector.tensor_tensor(bvg[:, 0, :], bv[:, 0, :], gamma[:], op=ALU.mult)
            nc.vector.tensor_tensor(bvg[:, 1, :], bv[:, 1, :], gamma[:], op=ALU.mult)
            for t in range(2):
                nc.vector.tensor_scalar_add(sem[t][:], sem[t][:], bvg[:, t, :])

            # ---- main attention loop over n-chunks ----
            for j in range(NCHUNKS):
                ns = slice(j * NCHUNK, (j + 1) * NCHUNK)
                pt = big.tile([P, MTILES // 2, 2 * NCHUNK], bf16, tag="big",
                              name="pt")
                y_stage = evict.tile([P, 2, NCHUNK], f32, tag="ystage",
                                     name="ystage", bufs=2)
                for mp in range(MTILES // 2):
                    st = st_ps.tile([P, 2 * NCHUNK], f32, tag="st", name="st")
                    m0, m1 = 2 * mp, 2 * mp + 1
                    nc.tensor.matmul(
                        st[:, 0:NCHUNK],
                        k2[0:D4, m0 * P:(m0 + 1) * P], q2[0:D4, ns],
                        start=True, stop=True, tile_position=(0, 0))
                    nc.tensor.matmul(
                        st[:, NCHUNK:2 * NCHUNK],
                        k2[D4:P, m1 * P:(m1 + 1) * P], q2[D4:P, ns],
                        start=True, stop=True, tile_position=(64, 0))
                    nc.scalar.activation(pt[:, mp, :], st[:], AF.Exp, scale=0.125)

                for t in range(NCHUNK // P):
                    oacc = oacc_ps.tile([P, DIM + 1], f32, tag="oacc", name="oacc")
                    for m in range(MTILES):
                        off = (m % 2) * NCHUNK + t * P
                        nc.tensor.matmul(
                            oacc[:],
                            pt[:, m // 2, off:off + P],
                            vt_ext[:, m, 0:DIM + 1],
                            start=(m == 0), stop=(m == MTILES - 1))
                    rinv = evict.tile([P, 1], f32, tag="rinv", name="rinv")
                    nc.vector.reciprocal(rinv[:], oacc[:, DIM:DIM + 1])
                    onrm = evict.tile([P, DIM], bf16, tag="onrm", name="onrm")
                    nc.vector.tensor_scalar_mul(onrm[:], oacc[:, 0:DIM], rinv[:])
                    gs = slice(j * NCHUNK + t * P, j * NCHUNK + (t + 1) * P)
                    for ct in range(2):
                        tp = tp_ps.tile([P, P], bf16, tag="tp", name="tp")
                        nc.tensor.transpose(tp[:], onrm[:, ct * P:(ct + 1) * P],
                                            identity_bf[:])
                        nc.vector.tensor_tensor(
                            y_stage[:, ct, t * P:(t + 1) * P], tp[:],
                            sem[ct][:, gs], op=ALU.add)
                for ct in range(2):
                    nc.scalar.dma_start(y_d[ct * P:(ct + 1) * P, ns],
                                        y_stage[:, ct, :])

    nc.compile()
    return nc


def _get_program():
    if "nc" not in _CACHE:
        _CACHE["nc"] = _build_program()
    return _CACHE["nc"]


def kernel(sem, foren, Wq, bq, Wk, bk, Wv, bv, gamma):
    sem = np.asarray(sem, dtype=np.float32)
    foren = np.asarray(foren, dtype=np.float32)
    wqt = np.ascontiguousarray(np.asarray(Wq, np.float32).T)
    wkt = np.ascontiguousarray(np.asarray(Wk, np.float32).T)
    wvt = np.ascontiguousarray(np.asarray(Wv, np.float32).T)
    bq2 = np.ascontiguousarray(np.tile(np.asarray(bq, np.float32).reshape(D4, 1), (2, 1)))
    bk2 = np.ascontiguousarray(np.tile(np.asarray(bk, np.float32).reshape(D4, 1), (2, 1)))
    bv2 = np.ascontiguousarray(np.asarray(bv, np.float32).reshape(DIM, 1))
    g2 = np.ascontiguousarray(np.asarray(gamma, np.float32).reshape(1, 1))

    B = sem.shape[0]
    assert B == N_CORES, f"expected batch {N_CORES}, got {B}"

    in_maps = []
    for i in range(N_CORES):
        in_maps.append({
            "sem_b": np.ascontiguousarray(sem[i].reshape(DIM, HW)),
            "foren_b": np.ascontiguousarray(foren[i].reshape(DIM, HW)),
            "WqT": wqt, "WkT": wkt, "WvT": wvt,
            "bq2": bq2, "bk2": bk2, "bv2": bv2, "gamma2": g2,
        })

    nc = _get_program()
    res = run_bass_kernel_spmd(nc, in_maps, list(range(N_CORES)), trace=TRACE)
    if TRACE:
        _CACHE["last_exec_time_ns"] = res.exec_time_ns
        _CACHE["last_results"] = res

    H = int(np.sqrt(HW))
    out = np.stack([res.results[i]["y"].reshape(DIM, H, H)
                    for i in range(N_CORES)])
    return out.astype(np.float32)


# revision 12
# speedup vs baseline: 1.2032x; 1.0090x over previous
"""Cross-attention kernel for Trainium2, data-parallel over batch across 8 NeuronCores.

Reference computation (per batch element b):
    q = Wq @ sem_b   [64, 4096]   (1x1 conv == per-pixel linear)
    k = Wk @ foren_b [64, 4096]
    v = Wv @ foren_b [256, 4096]
    S = (q^T k) / 8                [4096 (n), 4096 (m)]
    P = softmax_m(S)
    out = v @ P^T                  [256, 4096]
    y = sem_b + gamma * out

Kernel layout strategy (keys m live on partitions, so softmax needs no on-chip
transposes and the denominator rides the value matmul):
    - S^T[m, n] = sum_d k[d, m] q[d, n]: lhsT = k tile (K=64), rhs = q chunk.
      q/k are duplicated into partitions 64..127 so two K=64 matmuls run
      concurrently in the two halves of the PE array via tile_position.
    - P~ = exp(S^T/8) (no max subtraction: logits ~ N(0,1), exp is safe in fp32),
      ScalarE reads PSUM, writes bf16.
    - v^T[m, c] = sum_ch foren[ch, m] WvT[ch, c]: natural from PE; gamma folded
      into the eviction; a ones column is appended -> vt_ext [m, 257].
    - out^T[n, c'] = sum_m P~[m, n-slice] vt_ext[m, c']: lhsT = P~ slice
      (stationary), rhs = vt_ext. Column 256 of the accumulator is then
      sum_m P~[m, n] — the softmax denominator, for free.
    - Eviction: rinv = 1/acc[:, 256] (per-partition!), ScalarE Copy with
      scale=rinv normalizes while casting to bf16.
    - PE transpose (128x128, via identity) flips out^T back to [c, n];
      VectorE adds the residual sem + gamma*bv (bv folded there since
      out = v@P^T + bv when v carries bias).
"""

import os
import sys

for _p in ("/opt/trn_rl_repo",):
    if _p not in sys.path and os.path.isdir(_p):
        sys.path.append(_p)

import numpy as np

import concourse.bass as bass
import concourse.tile as tile
from concourse import bacc, mybir
from concourse.bass_utils import run_bass_kernel_spmd
from concourse.masks import make_identity

N_CORES = 8
DIM = 256
D4 = 64
HW = 4096
P = 128
NCHUNK = 512          # n per chunk
NCHUNKS = HW // NCHUNK
MTILES = HW // P      # 32 key tiles of 128
VTW = 264             # vt_ext row stride (257 used, padded for alignment)

TRACE = False         # test.py sets this to capture HW exec time
_CACHE = {}


def _build_program():
    f32 = mybir.dt.float32
    bf16 = mybir.dt.bfloat16
    AF = mybir.ActivationFunctionType
    ALU = mybir.AluOpType

    nc = bacc.Bacc("TRN2", target_bir_lowering=False, debug=False,
                   num_devices=N_CORES)

    sem_d = nc.dram_tensor("sem_b", [DIM, HW], f32, kind="ExternalInput")
    foren_d = nc.dram_tensor("foren_b", [DIM, HW], f32, kind="ExternalInput")
    wqt_d = nc.dram_tensor("WqT", [DIM, D4], f32, kind="ExternalInput")
    wkt_d = nc.dram_tensor("WkT", [DIM, D4], f32, kind="ExternalInput")
    wvt_d = nc.dram_tensor("WvT", [DIM, DIM], f32, kind="ExternalInput")
    bq_d = nc.dram_tensor("bq2", [P, 1], f32, kind="ExternalInput")
    bk_d = nc.dram_tensor("bk2", [P, 1], f32, kind="ExternalInput")
    bv_d = nc.dram_tensor("bv2", [DIM, 1], f32, kind="ExternalInput")
    gamma_d = nc.dram_tensor("gamma2", [1, 1], f32, kind="ExternalInput")
    y_d = nc.dram_tensor("y", [DIM, HW], f32, kind="ExternalOutput")

    with tile.TileContext(nc) as tc:
        with (
            tc.tile_pool(name="persist", bufs=1) as persist,
            tc.tile_pool(name="big", bufs=2) as big,        # foren_bf then P~ tiles
            tc.tile_pool(name="stage", bufs=3) as stage,    # fp32 DMA staging
            tc.tile_pool(name="evict", bufs=4) as evict,
            tc.tile_pool(name="st_ps", bufs=2, space="PSUM") as st_ps,
            tc.tile_pool(name="oacc_ps", bufs=2, space="PSUM") as oacc_ps,
            tc.tile_pool(name="tp_ps", bufs=2, space="PSUM") as tp_ps,
        ):
            # ---- chunked loads: sem on sync queue, foren on scalar queue;
            # casts and projections interleave per 512-column chunk so PE
            # starts projecting while later chunks are still in flight ----
            sem = [persist.tile([P, HW], f32, tag=f"sem{t}", name=f"sem{t}")
                   for t in range(2)]
            foren_bf = [big.tile([P, HW], bf16, tag="big", name=f"forenb{t}")
                        for t in range(2)]
            sem_bf = [persist.tile([P, HW], bf16, tag=f"semb{t}", name=f"semb{t}")
                      for t in range(2)]

            wqt = persist.tile([P, 2, D4], f32, tag="wqt")
            wkt = persist.tile([P, 2, D4], f32, tag="wkt")
            wvt = persist.tile([P, 2, DIM], f32, tag="wvt")
            nc.gpsimd.dma_start(wqt[:], wqt_d.ap().rearrange("(t p) o -> p t o", p=P))
            nc.gpsimd.dma_start(wkt[:], wkt_d.ap().rearrange("(t p) o -> p t o", p=P))
            nc.gpsimd.dma_start(wvt[:], wvt_d.ap().rearrange("(t p) o -> p t o", p=P))

            bq = persist.tile([P, 1], f32, tag="bq")
            bk = persist.tile([P, 1], f32, tag="bk")
            bv = persist.tile([P, 2, 1], f32, tag="bv")
            gamma = persist.tile([P, 1], f32, tag="gamma")
            nc.gpsimd.dma_start(bq[:], bq_d[:])
            nc.gpsimd.dma_start(bk[:], bk_d[:])
            nc.gpsimd.dma_start(bv[:], bv_d.ap().rearrange("(t p) o -> p t o", p=P))
            nc.gpsimd.dma_start(gamma[:], gamma_d.ap().to_broadcast([P, 1]))

            # ---- weight casts ----
            wqt_bf = persist.tile([P, 2, D4], bf16, tag="wqtb")
            wkt_bf = persist.tile([P, 2, D4], bf16, tag="wktb")
            wvt_bf = persist.tile([P, 2, DIM], bf16, tag="wvtb")
            nc.vector.tensor_copy(wqt_bf[:], wqt[:])
            nc.vector.tensor_copy(wkt_bf[:], wkt[:])
            nc.vector.tensor_copy(wvt_bf[:], wvt[:])

            identity_bf = persist.tile([P, P], bf16, tag="idn")
            make_identity(nc, identity_bf[:])

            # ---- q/k projections into lower partitions of q2/k2 ----
            q2 = persist.tile([P, HW], bf16, tag="q2")
            k2 = persist.tile([P, HW], bf16, tag="k2")
            vt_ext = persist.tile([P, MTILES, VTW], bf16, tag="vte")
            nc.vector.memset(vt_ext[:, :, DIM:DIM + 1], 1.0)
            # each projection matmul is issued twice with col-tiling so the
            # result lands in both partition halves of PSUM concurrently --
            # q2/k2 come out pre-duplicated for the row-packed QK matmuls
            for j in range(NCHUNKS):
                ns = slice(j * NCHUNK, (j + 1) * NCHUNK)
                for t in range(2):
                    nc.sync.dma_start(sem[t][:, ns], sem_d[t * P:(t + 1) * P, ns])
                    fst = stage.tile([P, NCHUNK], f32, tag="fst", name="fst")
                    nc.scalar.dma_start(fst[:], foren_d[t * P:(t + 1) * P, ns])
                    nc.vector.tensor_copy(foren_bf[t][:, ns], fst[:])
                    nc.scalar.copy(sem_bf[t][:, ns], sem[t][:, ns])
                pq = st_ps.tile([P, NCHUNK], f32, tag="st", name="pq")
                pk = st_ps.tile([P, NCHUNK], f32, tag="st", name="pk")
                for t in range(2):
                    for cg in range(2):
                        ps_ = slice(cg * D4, (cg + 1) * D4)
                        nc.tensor.matmul(pq[ps_, :], wqt_bf[:, t, :],
                                         sem_bf[t][:, ns],
                                         start=(t == 0), stop=(t == 1),
                                         tile_position=(0, cg * D4))
                        nc.tensor.matmul(pk[ps_, :], wkt_bf[:, t, :],
                                         foren_bf[t][:, ns],
                                         start=(t == 0), stop=(t == 1),
                                         tile_position=(0, cg * D4))
                nc.scalar.activation(q2[:, ns], pq[:], AF.Identity, bias=bq[:])
                nc.scalar.activation(k2[:, ns], pk[:], AF.Identity, bias=bk[:])
                # interleave vT projection (evicted on VectorE, own PSUM
                # rotation) so setup PSUM slots recycle through two engines
                for mi in range(4 * j, 4 * j + 4):
                    ms = slice(mi * P, (mi + 1) * P)
                    pv = oacc_ps.tile([P, DIM], f32, tag="oacc", name="pv")
                    for t in range(2):
                        nc.tensor.matmul(pv[:], foren_bf[t][:, ms],
                                         wvt_bf[:, t, :],
                                         start=(t == 0), stop=(t == 1))
                    nc.vector.tensor_scalar_mul(vt_ext[:, mi, 0:DIM], pv[:],
                                                gamma[:])

            # ---- residual base: sem_res = sem + gamma * bv (in place) ----
            bvg = persist.tile([P, 2, 1], f32, tag="bvg")
            nc.vector.tensor_tensor(bvg[:, 0, :], bv[:, 0, :], gamma[:], op=ALU.mult)
            nc.vector.tensor_tensor(bvg[:, 1, :], bv[:, 1, :], gamma[:], op=ALU.mult)
            for t in range(2):
                nc.vector.tensor_scalar_add(sem[t][:], sem[t][:], bvg[:, t, :])

            # ---- main attention loop over n-chunks ----
            for j in range(NCHUNKS):
                ns = slice(j * NCHUNK, (j + 1) * NCHUNK)
                pt = big.tile([P, MTILES // 2, 2 * NCHUNK], bf16, tag="big",
                              name="pt")
                y_stage = evict.tile([P, 2, NCHUNK], f32, tag="ystage",
                                     name="ystage", bufs=2)
                for mp in range(MTILES // 2):
                    st = st_ps.tile([P, 2 * NCHUNK], f32, tag="st", name="st")
                    m0, m1 = 2 * mp, 2 * mp + 1
                    nc.tensor.matmul(
                        st[:, 0:NCHUNK],
                        k2[0:D4, m0 * P:(m0 + 1) * P], q2[0:D4, ns],
                        start=True, stop=True, tile_position=(0, 0))
                    nc.tensor.matmul(
                        st[:, NCHUNK:2 * NCHUNK],
                        k2[D4:P, m1 * P:(m1 + 1) * P], q2[D4:P, ns],
                        start=True, stop=True, tile_position=(64, 0))
                    nc.scalar.activation(pt[:, mp, :], st[:], AF.Exp, scale=0.125)

                for t in range(NCHUNK // P):
                    oacc = oacc_ps.tile([P, DIM + 1], f32, tag="oacc", name="oacc")
                    for m in range(MTILES):
                        off = (m % 2) * NCHUNK + t * P
                        nc.tensor.matmul(
                            oacc[:],
                            pt[:, m // 2, off:off + P],
                            vt_ext[:, m, 0:DIM + 1],
                            start=(m == 0), stop=(m == MTILES - 1))
                    rinv = evict.tile([P, 1], f32, tag="rinv", name="rinv")
                    nc.vector.reciprocal(rinv[:], oacc[:, DIM:DIM + 1])
                    onrm = evict.tile([P, DIM], bf16, tag="onrm", name="onrm")
                    nc.vector.tensor_scalar_mul(onrm[:], oacc[:, 0:DIM], rinv[:])
                    gs = slice(j * NCHUNK + t * P, j * NCHUNK + (t + 1) * P)
                    for ct in range(2):
                        tp = tp_ps.tile([P, P], bf16, tag="tp", name="tp")
                        nc.tensor.transpose(tp[:], onrm[:, ct * P:(ct + 1) * P],
                                            identity_bf[:])
                        nc.vector.tensor_tensor(
                            y_stage[:, ct, t * P:(t + 1) * P], tp[:],
                            sem[ct][:, gs], op=ALU.add)
                for ct in range(2):
                    nc.scalar.dma_start(y_d[ct * P:(ct + 1) * P, ns],
                                        y_stage[:, ct, :])

    nc.compile()
    return nc


def _get_program():
    if "nc" not in _CACHE:
        _CACHE["nc"] = _build_program()
    return _CACHE["nc"]


def kernel(sem, foren, Wq, bq, Wk, bk, Wv, bv, gamma):
    sem = np.asarray(sem, dtype=np.float32)
    foren = np.asarray(foren, dtype=np.float32)
    wqt = np.ascontiguousarray(np.asarray(Wq, np.float32).T)
    wkt = np.ascontiguousarray(np.asarray(Wk, np.float32).T)
    wvt = np.ascontiguousarray(np.asarray(Wv, np.float32).T)
    bq2 = np.ascontiguousarray(np.tile(np.asarray(bq, np.float32).reshape(D4, 1), (2, 1)))
    bk2 = np.ascontiguousarray(np.tile(np.asarray(bk, np.float32).reshape(D4, 1), (2, 1)))
    bv2 = np.ascontiguousarray(np.asarray(bv, np.float32).reshape(DIM, 1))
    g2 = np.ascontiguousarray(np.asarray(gamma, np.float32).reshape(1, 1))

    B = sem.shape[0]
    assert B == N_CORES, f"expected batch {N_CORES}, got {B}"

    in_maps = []
    for i in range(N_CORES):
        in_maps.append({
            "sem_b": np.ascontiguousarray(sem[i].reshape(DIM, HW)),
            "foren_b": np.ascontiguousarray(foren[i].reshape(DIM, HW)),
            "WqT": wqt, "WkT": wkt, "WvT": wvt,
            "bq2": bq2, "bk2": bk2, "bv2": bv2, "gamma2": g2,
        })

    nc = _get_program()
    res = run_bass_kernel_spmd(nc, in_maps, list(range(N_CORES)), trace=TRACE)
    if TRACE:
        _CACHE["last_exec_time_ns"] = res.exec_time_ns
        _CACHE["last_results"] = res

    H = int(np.sqrt(HW))
    out = np.stack([res.results[i]["y"].reshape(DIM, H, H)
                    for i in range(N_CORES)])
    return out.astype(np.float32)
